# revision 8
# baseline (speedup 1.0000x reference)
"""Compressible Ogden strain-energy kernel for Trainium2 (Bass/Tile), 8-core SPMD.

Per quadrature point (reference):
  C  = F^T F;  J = sqrt(det C);  Cb = J^(-2/3) C;  lamb = eigvals(Cb)
  W  = sum_k mu_k/alpha_k (sum_i lamb_i^(alpha_k/2) - 3)
     + KAPPA/BETA^2 (J^BETA - BETA ln J - 1)

Device-side recipe (all elementwise fp32 over [128, T] SBUF planes):
  - invariants q = tr(C)/3, p2 = tr((C-qI)^2), ds = det(C-qI)
  - det C = q^3 + ds - q p2/2   (char-poly identity; one det expansion only)
  - eig(C) via trigonometric Cardano with the acos built from Arctan on the
    QUARTER angle (hw arctan domain is [-pi/2, pi/2]):
      h2 = cos(u/2) = sqrt((1+r)/2);  tan(u/4) = sqrt((1-h2)/(1+h2)) in [0,1]
      cos(u/3 + off) via Sin activation (args stay inside [-pi, pi])
  - eig(Cb) folded in log space: ln lamb = ln lamC - ln(detC)/3
  - powers: Exp(alpha_k/2 * ln lamb + ln|mu_k/alpha_k|)
  - W_vol = KAPPA/BETA^2 (detC - ln detC - 1)   (exact for BETA=2)

Layout: host pre-transposes each core's shard to column-major component
planes [128 part, 9 planes, T] so every on-chip access is contiguous
(strided 36B reads measured ~2x slower on both ACT and DVE).

Stage 1 (squares, products, plane-sums) runs per half-column so compute
overlaps the input DMA; everything downstream runs once at full T to halve
the per-instruction SBUF read-write-bubble overhead.

Engines: ACT takes all transcendentals/squares + affine folds (func(scale*x
+bias) eats a mult+add each); DVE takes binary ops with scalar_tensor_tensor
fusions; GPSIMD takes the independent plane-sum reductions; ACT instruction
order is pinned (add_dep_helper) so activation table sets load ~3x/kernel,
not 20x.
"""

import math

import numpy as np

import concourse.bacc as bacc
import concourse.mybir as mybir
import concourse.tile as tile
from concourse.bass_utils import run_bass_kernel_spmd
from concourse.tile import add_dep_helper

P = 128
NCORES = 8
KAPPA = 100.0
BETA = 2.0
F32 = mybir.dt.float32
AF = mybir.ActivationFunctionType
OP = mybir.AluOpType

RCLAMP = 1.0 - 1e-6
V_EPS = 1e-12
PI = math.pi


class Planes:
    """Contiguous-run plane allocator inside one big [P, NP*T] SBUF tile."""

    def __init__(self, ws, T, n):
        self.ws = ws
        self.T = T
        self.free_set = set(range(n))
        self.peak = 0
        self.n = n

    def alloc(self, k=1):
        free = sorted(self.free_set)
        run = None
        for i in range(len(free) - k + 1):
            if free[i + k - 1] - free[i] == k - 1:
                run = free[i]
                break
        if run is None:
            raise RuntimeError(f"no {k} contiguous planes free (free={free})")
        for j in range(run, run + k):
            self.free_set.remove(j)
        self.peak = max(self.peak, self.n - len(self.free_set))
        return run

    def release(self, base, k=1):
        for j in range(base, base + k):
            assert j not in self.free_set
            self.free_set.add(j)

    def ap(self, base, k=1, lo=None, hi=None):
        """AP over planes [base, base+k) columns [lo, hi) as [P, k, hi-lo]."""
        T = self.T
        full = self.ws[:, base * T:(base + k) * T]
        if lo is None:
            return full
        return full.rearrange("p (c t) -> p c t", c=k)[:, :, lo:hi]


class Emit:
    """Engine frontend that records ACT emission order for later pinning."""

    def __init__(self, nc):
        self.nc = nc
        self.acts = []

    def act(self, out, in_, func, bias=0.0, scale=1.0):
        i = self.nc.scalar.activation(out, in_, func, bias=bias, scale=scale)
        self.acts.append(i)
        return i

    def pin_act_order(self):
        for a, b in zip(self.acts, self.acts[1:]):
            add_dep_helper(b.ins, a.ins, sync=False, reason="act table-set order")


def build_nc(T, mu, alpha, debug=False, nplanes=44, halves=2, gp_sums=True):
    """Build the SPMD single-core program (identical program on all cores).

    T: points per partition (must be even; P*T points per core).
    halves: input DMA + stage-1 split into this many column slices.
    gp_sums: route independent plane-sum adds to GPSIMD.
    """
    assert T % (2 * halves) == 0
    mu64 = np.asarray(mu, np.float64)
    al64 = np.asarray(alpha, np.float64)
    alp2 = al64 * 0.5
    coef = mu64 / al64
    lncoef = [None if c == 0.0 else math.log(abs(c)) for c in coef]
    sgn = [0.0 if c == 0.0 else math.copysign(1.0, c) for c in coef]
    k0 = -KAPPA / (BETA * BETA) - 3.0 * float(np.sum(coef))

    nc = bacc.Bacc("TRN2", target_bir_lowering=False, debug=debug)

    # Register activation bias constants ([128,1] SBUF memsets, the same
    # pattern Bass.__init__ uses for 0.0/1.0).
    bias_vals = {math.log(0.5), PI / 2.0, -5.0 * PI / 6.0, 0.5, V_EPS}
    bias_vals.update(float(b) for b in lncoef if b is not None)
    for val in sorted(bias_vals):
        if (F32, val) in nc.const_aps.aps:
            continue
        tns = nc.alloc_sbuf_tensor(f"const-f32-{val!r}", [128, 1], F32)
        nc.gpsimd.memset(tns.ap(), val)
        nc.const_aps.aps[(F32, val)] = tns.ap()
    nc.all_engine_barrier()

    Fm = nc.dram_tensor("F", [P, 9 * T], F32, kind="ExternalInput")
    Wm = nc.dram_tensor("W", [P, T], F32, kind="ExternalOutput")
    Fv = Fm[:].rearrange("p (c t) -> p c t", c=9)

    H = T // halves

    with tile.TileContext(nc) as tc:
        with tc.tile_pool(name="ws", bufs=1) as pool:
            ws = pool.tile([P, nplanes * T], F32, tag="ws")
            pl = Planes(ws, T, nplanes)
            em = Emit(nc)
            vec = nc.vector
            gp = nc.gpsimd if gp_sums else nc.vector

            # ---- stage 1 per half-column: squares, products, plane sums ----
            ft = pl.alloc(9)   # col-major F planes: plane 3c+r = F[r][c]
            sq = pl.alloc(9)
            pr = pl.alloc(9)
            cd = pl.alloc(3)   # [c00 c11 c22]
            co = pl.alloc(3)   # [c01 c02 c12]
            t1 = pl.alloc(1)   # tr(C) = 3q
            for h in range(halves):
                lo, hi = h * H, (h + 1) * H
                nc.sync.dma_start(out=pl.ap(ft, 9, lo, hi), in_=Fv[:, :, lo:hi])
                em.act(pl.ap(sq, 9, lo, hi), pl.ap(ft, 9, lo, hi), AF.Square)
                colv = [pl.ap(ft + 3 * c, 3, lo, hi) for c in range(3)]
                vec.tensor_mul(pl.ap(pr + 0, 3, lo, hi), colv[0], colv[1])
                vec.tensor_mul(pl.ap(pr + 3, 3, lo, hi), colv[0], colv[2])
                vec.tensor_mul(pl.ap(pr + 6, 3, lo, hi), colv[1], colv[2])
                # c_xx = sum_r sq[3x+r]; cXY = sum_r pr[3g+r]  (r-th planes)
                sqr = [pl.ap(sq, 9, lo, hi).rearrange(
                    "p (x r) t -> p r x t", r=3)[:, r] for r in range(3)]
                prr = [pl.ap(pr, 9, lo, hi).rearrange(
                    "p (g r) t -> p r g t", r=3)[:, r] for r in range(3)]
                cdv = pl.ap(cd, 3, lo, hi)
                cov = pl.ap(co, 3, lo, hi)
                gp.tensor_add(cdv, sqr[0], sqr[1])
                gp.tensor_add(cdv, cdv, sqr[2])
                gp.tensor_add(cov, prr[0], prr[1])
                gp.tensor_add(cov, cov, prr[2])
                vec.tensor_add(pl.ap(t1, 1, lo, hi),
                               pl.ap(cd, 1, lo, hi), pl.ap(cd + 1, 1, lo, hi))
                vec.tensor_add(pl.ap(t1, 1, lo, hi),
                               pl.ap(t1, 1, lo, hi), pl.ap(cd + 2, 1, lo, hi))
            pl.release(ft, 9)
            pl.release(pr, 9)

            # ---- full-T from here on ----
            # D = C_diag - tr/3
            dd = pl.alloc(3)
            t1b = pl.ap(t1).unsqueeze(1).broadcast_to([P, 3, T])
            vec.scalar_tensor_tensor(
                pl.ap(dd, 3).rearrange("p (i t) -> p i t", i=3),
                t1b, -1.0 / 3.0,
                pl.ap(cd, 3).rearrange("p (i t) -> p i t", i=3),
                OP.mult, OP.add)
            pl.release(cd, 3)

            osq = pl.alloc(3)
            em.act(pl.ap(osq, 3), pl.ap(co, 3), AF.Square)
            dsq = pl.alloc(3)
            em.act(pl.ap(dsq, 3), pl.ap(dd, 3), AF.Square)

            # p2 = sum(dsq) + 2 sum(osq); lnv = Ln(p2/6 + eps)
            p1 = pl.alloc(1)
            vec.tensor_add(pl.ap(p1), pl.ap(osq), pl.ap(osq + 1))
            vec.tensor_add(pl.ap(p1), pl.ap(p1), pl.ap(osq + 2))
            sd = pl.alloc(1)
            vec.tensor_add(pl.ap(sd), pl.ap(dsq), pl.ap(dsq + 1))
            vec.tensor_add(pl.ap(sd), pl.ap(sd), pl.ap(dsq + 2))
            pl.release(dsq, 3)
            p2 = pl.alloc(1)
            vec.scalar_tensor_tensor(pl.ap(p2), pl.ap(p1), 2.0, pl.ap(sd),
                                     OP.mult, OP.add)
            pl.release(p1)
            pl.release(sd)
            lnv = pl.alloc(1)
            em.act(pl.ap(lnv), pl.ap(p2), AF.Ln, scale=1.0 / 6.0, bias=V_EPS)
            pp = pl.alloc(1)
            em.act(pl.ap(pp), pl.ap(lnv), AF.Exp, scale=0.5)        # p
            w = pl.alloc(1)
            em.act(pl.ap(w), pl.ap(lnv), AF.Exp, scale=-1.5,
                   bias=math.log(0.5))                              # 0.5 v^-1.5
            pl.release(lnv)

            # ds = det(C - qI)
            m1 = pl.alloc(1)
            vec.tensor_mul(pl.ap(m1), pl.ap(dd + 1), pl.ap(dd + 2))
            vec.tensor_sub(pl.ap(m1), pl.ap(m1), pl.ap(osq + 2))     # y1
            m2 = pl.alloc(1)
            vec.tensor_mul(pl.ap(m2), pl.ap(co), pl.ap(dd + 2))
            m3 = pl.alloc(1)
            vec.tensor_mul(pl.ap(m3), pl.ap(co + 1), pl.ap(co + 2))
            vec.tensor_sub(pl.ap(m2), pl.ap(m2), pl.ap(m3))          # y2
            m4 = pl.alloc(1)
            vec.tensor_mul(pl.ap(m4), pl.ap(co), pl.ap(co + 2))
            vec.tensor_mul(pl.ap(m3), pl.ap(co + 1), pl.ap(dd + 1))
            vec.tensor_sub(pl.ap(m4), pl.ap(m4), pl.ap(m3))          # y3
            pl.release(m3)
            pl.release(osq, 3)
            vec.tensor_mul(pl.ap(m1), pl.ap(dd), pl.ap(m1))
            vec.tensor_mul(pl.ap(m2), pl.ap(co), pl.ap(m2))
            vec.tensor_mul(pl.ap(m4), pl.ap(co + 1), pl.ap(m4))
            pl.release(co, 3)
            pl.release(dd, 3)
            vec.tensor_sub(pl.ap(m1), pl.ap(m1), pl.ap(m2))
            vec.tensor_add(pl.ap(m1), pl.ap(m1), pl.ap(m4))          # ds
            pl.release(m2)
            pl.release(m4)
            ds = m1

            # detC = (tr/3)^3 + ds - 0.5 (tr/3) p2;   t = Ln(detC)
            qsq = pl.alloc(1)
            em.act(pl.ap(qsq), pl.ap(t1), AF.Square, scale=1.0 / 3.0)  # q^2
            vec.scalar_tensor_tensor(pl.ap(qsq), pl.ap(t1), 1.0 / 3.0,
                                     pl.ap(qsq), OP.mult, OP.mult)     # q^3
            qp2 = pl.alloc(1)
            vec.scalar_tensor_tensor(pl.ap(qp2), pl.ap(t1), 1.0 / 3.0,
                                     pl.ap(p2), OP.mult, OP.mult)      # q p2
            pl.release(p2)
            vec.tensor_add(pl.ap(qsq), pl.ap(qsq), pl.ap(ds))
            detc = pl.alloc(1)
            vec.scalar_tensor_tensor(pl.ap(detc), pl.ap(qp2), -0.5,
                                     pl.ap(qsq), OP.mult, OP.add)
            pl.release(qsq)
            pl.release(qp2)
            tt = pl.alloc(1)
            em.act(pl.ap(tt), pl.ap(detc), AF.Ln)

            # r = ds * w clamped; tan(u/4) = sqrt((1-h2)/(1+h2))
            vec.tensor_mul(pl.ap(ds), pl.ap(ds), pl.ap(w))
            pl.release(w)
            rc = ds
            vec.tensor_scalar(pl.ap(rc), pl.ap(rc), -RCLAMP, RCLAMP,
                              OP.max, OP.min)
            la = pl.alloc(1)
            em.act(pl.ap(la), pl.ap(rc), AF.Ln, scale=0.5, bias=0.5)
            pl.release(rc)
            h2 = pl.alloc(1)
            em.act(pl.ap(h2), pl.ap(la), AF.Exp, scale=0.5)          # cos(u/2)
            lnm = la
            em.act(pl.ap(lnm), pl.ap(h2), AF.Ln, scale=-1.0, bias=1.0)
            lnp = pl.alloc(1)
            em.act(pl.ap(lnp), pl.ap(h2), AF.Ln, scale=1.0, bias=1.0)
            pl.release(h2)
            vec.tensor_sub(pl.ap(lnm), pl.ap(lnm), pl.ap(lnp))
            pl.release(lnp)
            xt = lnm
            em.act(pl.ap(xt), pl.ap(xt), AF.Exp, scale=0.5)          # tan(u/4)

            # ---- trig set: u/4 = Arctan; cos terms via Sin ----
            em.act(pl.ap(xt), pl.ap(xt), AF.Arctan)
            ar = xt
            c1 = pl.alloc(1)
            em.act(pl.ap(c1), pl.ap(ar), AF.Sin, scale=4.0 / 3.0, bias=PI / 2.0)
            c2 = pl.alloc(1)
            em.act(pl.ap(c2), pl.ap(ar), AF.Sin, scale=4.0 / 3.0,
                   bias=-5.0 * PI / 6.0)
            pl.release(ar)

            # lamC: lam0/2 = q + 2 p cos; lam1 = 3q - lam0 - lam2
            lam = pl.alloc(3)
            vec.scalar_tensor_tensor(pl.ap(lam), pl.ap(c1), 2.0, pl.ap(pp),
                                     OP.mult, OP.mult)
            vec.scalar_tensor_tensor(pl.ap(lam), pl.ap(t1), 1.0 / 3.0,
                                     pl.ap(lam), OP.mult, OP.add)
            vec.scalar_tensor_tensor(pl.ap(lam + 2), pl.ap(c2), 2.0, pl.ap(pp),
                                     OP.mult, OP.mult)
            vec.scalar_tensor_tensor(pl.ap(lam + 2), pl.ap(t1), 1.0 / 3.0,
                                     pl.ap(lam + 2), OP.mult, OP.add)
            vec.tensor_sub(pl.ap(lam + 1), pl.ap(t1), pl.ap(lam))
            vec.tensor_sub(pl.ap(lam + 1), pl.ap(lam + 1), pl.ap(lam + 2))
            pl.release(c1)
            pl.release(c2)
            pl.release(pp)
            pl.release(t1)

            # ---- back to ln/exp set: powers and assembly ----
            em.act(pl.ap(lam, 3), pl.ap(lam, 3), AF.Ln)
            lnl = lam
            ttb = pl.ap(tt).unsqueeze(1).broadcast_to([P, 3, T])
            lnl3 = pl.ap(lnl, 3).rearrange("p (i t) -> p i t", i=3)
            vec.scalar_tensor_tensor(lnl3, ttb, -1.0 / 3.0, lnl3,
                                     OP.mult, OP.add)

            ee = pl.alloc(9)
            live_k = [k for k in range(3) if lncoef[k] is not None]
            for k in live_k:
                em.act(pl.ap(ee + 3 * k, 3), pl.ap(lnl, 3), AF.Exp,
                       scale=float(alp2[k]), bias=float(lncoef[k]))
            pl.release(lnl, 3)

            pw = pl.alloc(3)
            egr = [pl.ap(ee, 9).rearrange("p (k i t) -> p i k t", k=3, i=3)[:, i]
                   for i in range(3)]
            pw3 = pl.ap(pw, 3).rearrange("p (k t) -> p k t", k=3)
            gp.tensor_add(pw3, egr[0], egr[1])
            gp.tensor_add(pw3, pw3, egr[2])
            pl.release(ee, 9)
            for k in live_k:
                if sgn[k] < 0:
                    vec.tensor_scalar(pl.ap(pw + k), pl.ap(pw + k), -1.0, None,
                                      OP.mult)

            # acc = sum_k sgn_k pw_k + k0;  W = acc + K/B^2 (detC - t)
            acc = pl.alloc(1)
            ks = live_k or []
            if not ks:
                nc.vector.memset(pl.ap(acc), float(k0))
            elif len(ks) == 1:
                vec.tensor_scalar(pl.ap(acc), pl.ap(pw + ks[0]), float(k0),
                                  None, OP.add)
            else:
                vec.tensor_add(pl.ap(acc), pl.ap(pw + ks[0]), pl.ap(pw + ks[1]))
                for k in ks[2:-1]:
                    vec.tensor_add(pl.ap(acc), pl.ap(acc), pl.ap(pw + k))
                vec.scalar_tensor_tensor(pl.ap(acc), pl.ap(pw + ks[-1]),
                                         float(k0), pl.ap(acc), OP.add, OP.add)
            pl.release(pw, 3)
            vec.tensor_sub(pl.ap(detc), pl.ap(detc), pl.ap(tt))
            pl.release(tt)
            wout = pl.alloc(1)
            vec.scalar_tensor_tensor(pl.ap(wout), pl.ap(detc),
                                     KAPPA / (BETA * BETA), pl.ap(acc),
                                     OP.mult, OP.add)
            pl.release(detc)
            pl.release(acc)
            nc.sync.dma_start(out=Wm[:, :], in_=pl.ap(wout))
            pl.release(wout)

            em.pin_act_order()
    nc.compile()
    return nc


def _pad_and_shard(F, T):
    """-> [NCORES, P, 9T] col-major component planes (c-major, r-minor)."""
    n = F.shape[0]
    per_core = P * T
    npad = NCORES * per_core
    flat = np.ascontiguousarray(F, dtype=np.float32).reshape(n, 9)
    if npad > n:
        pad = np.tile(np.eye(3, dtype=np.float32).reshape(1, 9), (npad - n, 1))
        flat = np.concatenate([flat, pad], axis=0)
    a = flat.reshape(NCORES, P, T, 3, 3)          # [.., t, r, c]
    a = np.ascontiguousarray(a.transpose(0, 1, 4, 3, 2))  # [.., c, r, t]
    return a.reshape(NCORES, P, 9 * T)


def kernel(F, mu, alpha):
    F = np.asarray(F)
    n = F.shape[0]
    T = -(-n // (NCORES * P))
    T += (-T) % 4                      # halves stay even for DVE 2x mode
    shards = _pad_and_shard(F, T)
    nc = build_nc(T, mu, alpha)
    in_maps = [{"F": shards[i]} for i in range(NCORES)]
    res = run_bass_kernel_spmd(nc, in_maps, list(range(NCORES)))
    out = np.concatenate([res.results[i]["W"].reshape(-1) for i in range(NCORES)])
    return out[:n].astype(np.float32, copy=False)


if __name__ == "__main__":
    rng = np.random.default_rng(0)
    F = np.eye(3, dtype=np.float32) + 0.1 * rng.standard_normal((4096, 3, 3)).astype(np.float32)
    mu = np.array([0.63, 0.0012, -0.01], np.float32)
    alpha = np.array([1.3, 5.0, -2.0], np.float32)
    print(kernel(F, mu, alpha)[:8])


# revision 10
# speedup vs baseline: 1.1511x; 1.1511x over previous
"""Compressible Ogden strain-energy kernel for Trainium2 (Bass/Tile), 8-core SPMD.

Per quadrature point (reference):
  C  = F^T F;  J = sqrt(det C);  Cb = J^(-2/3) C;  lamb = eigvals(Cb)
  W  = sum_k mu_k/alpha_k (sum_i lamb_i^(alpha_k/2) - 3)
     + KAPPA/BETA^2 (J^BETA - BETA ln J - 1)

Device recipe (elementwise fp32 over [128, Tc] SBUF planes):
  - invariants q = tr(C)/3, p2 = tr((C-qI)^2), ds = det(C-qI)
  - det C = q^3 + ds - q p2/2            (char-poly identity)
  - eig(C) by trigonometric Cardano; acos from Arctan on the QUARTER angle
    (hw arctan domain is [-pi/2, pi/2]):
      h2 = cos(u/2) = sqrt((1+r)/2);  tan(u/4) = sqrt((1-h2)/(1+h2)) in [0,1]
      cos(u/3 + off) via Sin (args stay inside [-pi, pi])
  - eig(Cb) folded in log space: ln lamb = ln lamC - ln(detC)/3
  - powers: Exp(alpha_k/2 * ln lamb + ln|mu_k/alpha_k|)
  - W_vol = KAPPA/BETA^2 (detC - ln detC - 1)    (exact for BETA=2)

Performance structure (measured on hw):
  - host pre-transposes shards to column-major component planes so every
    on-chip access is contiguous (strided 36B reads are ~2x slower)
  - 2 column-chunks pipelined stage-major so DVE work of one chunk overlaps
    ACT work of the other; chunk FD kept >= 512 (smaller DVE ops pay an
    extra ~300-cycle inter-instruction SBUF bubble)
  - activation table sets here are per-function (ln/exp/arctan/sin all
    separate); ACT order is pinned (add_dep_helper) with both chunks'
    same-function calls adjacent -> ~11 table loads instead of ~20
  - no GPSIMD: it shares an SBUF port with DVE; concurrent gpsimd
    tensor ops measurably stall DVE 2-3x
  - scalar_tensor_tensor fuses (x op s) op y; activation fuses
    func(scale*x + bias); paired planes are placed adjacent so many ops
    process 2-3 planes per instruction
"""

import math

import numpy as np

import concourse.bacc as bacc
import concourse.mybir as mybir
import concourse.tile as tile
from concourse.bass_utils import run_bass_kernel_spmd
from concourse.tile import add_dep_helper

P = 128
NCORES = 8
KAPPA = 100.0
BETA = 2.0
F32 = mybir.dt.float32
AF = mybir.ActivationFunctionType
OP = mybir.AluOpType

RCLAMP = 1.0 - 1e-6
V_EPS = 1e-12
PI = math.pi


class Planes:
    """Contiguous-run plane allocator inside one big [P, NP*Tc] SBUF tile."""

    def __init__(self, ws, T, n):
        self.ws = ws
        self.T = T
        self.free_set = set(range(n))
        self.peak = 0
        self.n = n

    def alloc(self, k=1):
        free = sorted(self.free_set)
        run = None
        for i in range(len(free) - k + 1):
            if free[i + k - 1] - free[i] == k - 1:
                run = free[i]
                break
        if run is None:
            raise RuntimeError(f"no {k} contiguous planes free (free={free})")
        for j in range(run, run + k):
            self.free_set.remove(j)
        self.peak = max(self.peak, self.n - len(self.free_set))
        return run

    def release(self, base, k=1):
        for j in range(base, base + k):
            assert j not in self.free_set
            self.free_set.add(j)

    def ap(self, base, k=1):
        T = self.T
        return self.ws[:, base * T:(base + k) * T]

    def ap3(self, base, k=1):
        return self.ap(base, k).rearrange("p (c t) -> p c t", c=k)


class Emit:
    """Records ACT emission order for pinning (keeps table-set batching)."""

    def __init__(self, nc):
        self.nc = nc
        self.acts = []

    def act(self, out, in_, func, bias=0.0, scale=1.0):
        i = self.nc.scalar.activation(out, in_, func, bias=bias, scale=scale)
        self.acts.append(i)
        return i

    def pin_act_order(self):
        for a, b in zip(self.acts, self.acts[1:]):
            add_dep_helper(b.ins, a.ins, sync=False, reason="act table-set order")


def build_nc(T, mu, alpha, debug=False, nplanes=38, chunks=2):
    """Build the SPMD single-core program (identical on all cores).

    T points per partition per core; split into `chunks` column-chunks.
    """
    assert T % (2 * chunks) == 0
    Tc = T // chunks
    mu64 = np.asarray(mu, np.float64)
    al64 = np.asarray(alpha, np.float64)
    alp2 = al64 * 0.5
    coef = mu64 / al64
    lncoef = [None if c == 0.0 else math.log(abs(c)) for c in coef]
    sgn = [0.0 if c == 0.0 else math.copysign(1.0, c) for c in coef]
    k0 = -KAPPA / (BETA * BETA) - 3.0 * float(np.sum(coef))
    live_k = [k for k in range(3) if lncoef[k] is not None]

    nc = bacc.Bacc("TRN2", target_bir_lowering=False, debug=debug)

    bias_vals = {math.log(0.5), PI / 2.0, -5.0 * PI / 6.0, 0.5, 1.0, V_EPS}
    bias_vals.update(float(b) for b in lncoef if b is not None)
    for val in sorted(bias_vals):
        if (F32, val) in nc.const_aps.aps:
            continue
        tns = nc.alloc_sbuf_tensor(f"const-f32-{val!r}", [128, 1], F32)
        nc.gpsimd.memset(tns.ap(), val)
        nc.const_aps.aps[(F32, val)] = tns.ap()
    nc.all_engine_barrier()

    Fm = nc.dram_tensor("F", [P, 9 * T], F32, kind="ExternalInput")
    Wm = nc.dram_tensor("W", [P, T], F32, kind="ExternalOutput")
    Fv = Fm[:].rearrange("p (c t) -> p c t", c=9)

    def bc(ap2, k):
        return ap2.unsqueeze(1).broadcast_to([P, k, ap2.shape[-1]])

    with tile.TileContext(nc) as tc:
        with tc.tile_pool(name="ws", bufs=1) as pool:
            em = Emit(nc)
            vec = nc.vector
            pls, sts = [], []
            for ch in range(chunks):
                ws = pool.tile([P, nplanes * Tc], F32, tag=f"ws{ch}")
                pls.append(Planes(ws, Tc, nplanes))
                sts.append({})

            def s0_load_c(ch):
                """DMA in; squares of F; column products; C plane-sums."""
                pl, st = pls[ch], sts[ch]
                ft = pl.alloc(9)
                nc.sync.dma_start(out=pl.ap3(ft, 9),
                                  in_=Fv[:, :, ch * Tc:(ch + 1) * Tc])
                sq = pl.alloc(9)
                em.act(pl.ap(sq, 9), pl.ap(ft, 9), AF.Square)
                pr = pl.alloc(9)
                colv = [pl.ap(ft + 3 * c, 3) for c in range(3)]
                vec.tensor_mul(pl.ap(pr + 0, 3), colv[0], colv[1])
                vec.tensor_mul(pl.ap(pr + 3, 3), colv[0], colv[2])
                vec.tensor_mul(pl.ap(pr + 6, 3), colv[1], colv[2])
                pl.release(ft, 9)
                cd = pl.alloc(3)
                dd6 = pl.alloc(6)   # [d0 d1 d2 | c01 c02 c12]
                co = dd6 + 3
                sqr = pl.ap3(sq, 9).rearrange("p (x r) t -> p r x t", r=3)
                vec.tensor_add(pl.ap3(cd, 3), sqr[:, 0], sqr[:, 1])
                vec.tensor_add(pl.ap3(cd, 3), pl.ap3(cd, 3), sqr[:, 2])
                pl.release(sq, 9)
                prr = pl.ap3(pr, 9).rearrange("p (g r) t -> p r g t", r=3)
                vec.tensor_add(pl.ap3(co, 3), prr[:, 0], prr[:, 1])
                vec.tensor_add(pl.ap3(co, 3), pl.ap3(co, 3), prr[:, 2])
                pl.release(pr, 9)
                t1 = pl.alloc(1)
                vec.tensor_add(pl.ap(t1), pl.ap(cd), pl.ap(cd + 1))
                vec.tensor_add(pl.ap(t1), pl.ap(t1), pl.ap(cd + 2))
                st.update(cd=cd, dd6=dd6, t1=t1)

            def s1_invar(ch):
                """Deviatoric diag, squares, p2 = sum(d^2) + 2 sum(off^2)."""
                pl, st = pls[ch], sts[ch]
                cd, dd6, t1 = st["cd"], st["dd6"], st["t1"]
                vec.scalar_tensor_tensor(
                    pl.ap3(dd6, 3), bc(pl.ap(t1), 3), -1.0 / 3.0,
                    pl.ap3(cd, 3), OP.mult, OP.add)
                pl.release(cd, 3)
                sqb = pl.alloc(6)   # [d^2(3) | off^2(3)]
                em.act(pl.ap(sqb, 6), pl.ap(dd6, 6), AF.Square)
                psd = pl.alloc(2)   # [sd, p1]
                pairs = pl.ap3(sqb, 6).rearrange("p (y x) t -> p x y t", y=2)
                vec.tensor_add(pl.ap3(psd, 2), pairs[:, 0], pairs[:, 1])
                vec.tensor_add(pl.ap3(psd, 2), pl.ap3(psd, 2), pairs[:, 2])
                p2 = pl.alloc(1)
                vec.scalar_tensor_tensor(pl.ap(p2), pl.ap(psd + 1), 2.0,
                                         pl.ap(psd), OP.mult, OP.add)
                pl.release(psd, 2)
                st.update(sqb=sqb, p2=p2)

            def s2_lnv(ch):
                pl, st = pls[ch], sts[ch]
                lnv = pl.alloc(1)
                em.act(pl.ap(lnv), pl.ap(st["p2"]), AF.Ln,
                       scale=1.0 / 6.0, bias=V_EPS)
                st["lnv"] = lnv

            def s3_dets(ch):
                """ds = det(C - qI); detC = q^3 + ds - q p2/2."""
                pl, st = pls[ch], sts[ch]
                dd6, t1, p2, sqb = st["dd6"], st["t1"], st["p2"], st["sqb"]
                dd6a = pl.ap3(dd6, 6)
                g1 = pl.alloc(2)    # [d1*d2, c01*d2]
                vec.tensor_mul(pl.ap3(g1, 2), dd6a[:, 1:4:2],
                               bc(pl.ap(dd6 + 2), 2))
                g2 = pl.alloc(2)    # [c01*c12, c02*c12]
                vec.tensor_mul(pl.ap3(g2, 2), dd6a[:, 3:5],
                               bc(pl.ap(dd6 + 5), 2))
                g3 = pl.alloc(1)    # c02*d1
                vec.tensor_mul(pl.ap(g3), pl.ap(dd6 + 4), pl.ap(dd6 + 1))
                yb = pl.alloc(3)
                vec.tensor_sub(pl.ap(yb), pl.ap(g1), pl.ap(sqb + 5))
                vec.tensor_sub(pl.ap(yb + 1), pl.ap(g1 + 1), pl.ap(g2 + 1))
                vec.tensor_sub(pl.ap(yb + 2), pl.ap(g2), pl.ap(g3))
                pl.release(g1, 2)
                pl.release(g2, 2)
                pl.release(g3)
                pl.release(sqb, 6)
                zb = pl.alloc(3)
                vec.tensor_mul(pl.ap(zb), pl.ap(dd6), pl.ap(yb))
                vec.tensor_mul(pl.ap3(zb + 1, 2), dd6a[:, 3:5],
                               pl.ap3(yb + 1, 2))
                pl.release(yb, 3)
                pl.release(dd6, 6)
                ds = pl.alloc(1)
                vec.tensor_sub(pl.ap(ds), pl.ap(zb), pl.ap(zb + 1))
                vec.tensor_add(pl.ap(ds), pl.ap(ds), pl.ap(zb + 2))
                pl.release(zb, 3)
                qsq = pl.alloc(1)
                em.act(pl.ap(qsq), pl.ap(t1), AF.Square, scale=1.0 / 3.0)
                vec.scalar_tensor_tensor(pl.ap(qsq), pl.ap(t1), 1.0 / 3.0,
                                         pl.ap(qsq), OP.mult, OP.mult)  # q^3
                qp2 = pl.alloc(1)
                vec.scalar_tensor_tensor(pl.ap(qp2), pl.ap(t1), 1.0 / 3.0,
                                         pl.ap(p2), OP.mult, OP.mult)
                pl.release(p2)
                vec.tensor_add(pl.ap(qsq), pl.ap(qsq), pl.ap(ds))
                detc = pl.alloc(1)
                vec.scalar_tensor_tensor(pl.ap(detc), pl.ap(qp2), -0.5,
                                         pl.ap(qsq), OP.mult, OP.add)
                pl.release(qsq)
                pl.release(qp2)
                st.update(ds=ds, detc=detc)

            def s4_pw_exp(ch):
                pl, st = pls[ch], sts[ch]
                lnv = st.pop("lnv")
                pp = pl.alloc(1)
                em.act(pl.ap(pp), pl.ap(lnv), AF.Exp, scale=0.5)
                w = pl.alloc(1)
                em.act(pl.ap(w), pl.ap(lnv), AF.Exp, scale=-1.5,
                       bias=math.log(0.5))
                pl.release(lnv)
                st.update(p=pp, w=w)

            def s5_rc(ch):
                pl, st = pls[ch], sts[ch]
                ds, w = st.pop("ds"), st.pop("w")
                vec.tensor_mul(pl.ap(ds), pl.ap(ds), pl.ap(w))
                pl.release(w)
                vec.tensor_scalar(pl.ap(ds), pl.ap(ds), -RCLAMP, RCLAMP,
                                  OP.max, OP.min)
                st["rc"] = ds

            def s6_ln_a(ch):
                pl, st = pls[ch], sts[ch]
                rc = st.pop("rc")
                la = pl.alloc(1)
                em.act(pl.ap(la), pl.ap(rc), AF.Ln, scale=0.5, bias=0.5)
                pl.release(rc)
                tt = pl.alloc(1)
                em.act(pl.ap(tt), pl.ap(st["detc"]), AF.Ln)
                st.update(la=la, t=tt)

            def s7_h2(ch):
                pl, st = pls[ch], sts[ch]
                la = st.pop("la")
                h2 = pl.alloc(1)
                em.act(pl.ap(h2), pl.ap(la), AF.Exp, scale=0.5)
                pl.release(la)
                st["h2"] = h2

            def s8_ln_b(ch):
                pl, st = pls[ch], sts[ch]
                h2 = st.pop("h2")
                lnm = pl.alloc(1)
                em.act(pl.ap(lnm), pl.ap(h2), AF.Ln, scale=-1.0, bias=1.0)
                lnp = pl.alloc(1)
                em.act(pl.ap(lnp), pl.ap(h2), AF.Ln, scale=1.0, bias=1.0)
                pl.release(h2)
                st.update(lnm=lnm, lnp=lnp)

            def s9_sub(ch):
                pl, st = pls[ch], sts[ch]
                lnm, lnp = st.pop("lnm"), st.pop("lnp")
                vec.tensor_sub(pl.ap(lnm), pl.ap(lnm), pl.ap(lnp))
                pl.release(lnp)
                st["df"] = lnm

            def s10_xt(ch):
                pl, st = pls[ch], sts[ch]
                df = st["df"]
                em.act(pl.ap(df), pl.ap(df), AF.Exp, scale=0.5)  # tan(u/4)

            def s11_atan(ch):
                pl, st = pls[ch], sts[ch]
                df = st["df"]
                em.act(pl.ap(df), pl.ap(df), AF.Arctan)          # u/4

            def s12_sin(ch):
                pl, st = pls[ch], sts[ch]
                ar = st.pop("df")
                cb = pl.alloc(2)
                em.act(pl.ap(cb), pl.ap(ar), AF.Sin, scale=4.0 / 3.0,
                       bias=PI / 2.0)
                em.act(pl.ap(cb + 1), pl.ap(ar), AF.Sin, scale=4.0 / 3.0,
                       bias=-5.0 * PI / 6.0)
                pl.release(ar)
                st["cb"] = cb

            def s13_lam(ch):
                pl, st = pls[ch], sts[ch]
                cb, pp, t1 = st.pop("cb"), st.pop("p"), st.pop("t1")
                vec.scalar_tensor_tensor(pl.ap3(cb, 2), pl.ap3(cb, 2), 2.0,
                                         bc(pl.ap(pp), 2), OP.mult, OP.mult)
                pl.release(pp)
                lam = pl.alloc(3)
                lam3 = pl.ap3(lam, 3)
                vec.scalar_tensor_tensor(lam3[:, 0:3:2], bc(pl.ap(t1), 2),
                                         1.0 / 3.0, pl.ap3(cb, 2),
                                         OP.mult, OP.add)
                pl.release(cb, 2)
                vec.tensor_sub(pl.ap(lam + 1), pl.ap(t1), pl.ap(lam))
                pl.release(t1)
                vec.tensor_sub(pl.ap(lam + 1), pl.ap(lam + 1), pl.ap(lam + 2))
                # y = detC - t while DVE has the slot (W_vol argument)
                detc, tt = st.pop("detc"), st["t"]
                vec.tensor_sub(pl.ap(detc), pl.ap(detc), pl.ap(tt))
                st.update(lam=lam, y=detc)

            def s14_lnl(ch):
                pl, st = pls[ch], sts[ch]
                lam = st["lam"]
                em.act(pl.ap(lam, 3), pl.ap(lam, 3), AF.Ln)

            def s15_lp(ch):
                pl, st = pls[ch], sts[ch]
                lam, tt = st["lam"], st.pop("t")
                lnl3 = pl.ap3(lam, 3)
                vec.scalar_tensor_tensor(lnl3, bc(pl.ap(tt), 3), -1.0 / 3.0,
                                         lnl3, OP.mult, OP.add)
                pl.release(tt)

            def s16_exp(ch):
                pl, st = pls[ch], sts[ch]
                lam = st.pop("lam")
                ee = pl.alloc(9)
                for k in live_k:
                    em.act(pl.ap(ee + 3 * k, 3), pl.ap(lam, 3), AF.Exp,
                           scale=float(alp2[k]), bias=float(lncoef[k]))
                pl.release(lam, 3)
                st["ee"] = ee

            def s17_tail(ch):
                pl, st = pls[ch], sts[ch]
                ee, y = st.pop("ee"), st.pop("y")
                pw = pl.alloc(3)
                egr = pl.ap3(ee, 9).rearrange("p (k i) t -> p i k t", i=3)
                pw3 = pl.ap3(pw, 3)
                vec.tensor_add(pw3, egr[:, 0], egr[:, 1])
                vec.tensor_add(pw3, pw3, egr[:, 2])
                pl.release(ee, 9)
                for k in live_k:
                    if sgn[k] < 0:
                        vec.tensor_scalar(pl.ap(pw + k), pl.ap(pw + k), -1.0,
                                          None, OP.mult)
                acc = pl.alloc(1)
                ks = live_k
                if not ks:
                    nc.vector.memset(pl.ap(acc), float(k0))
                elif len(ks) == 1:
                    vec.tensor_scalar(pl.ap(acc), pl.ap(pw + ks[0]), float(k0),
                                      None, OP.add)
                else:
                    vec.tensor_add(pl.ap(acc), pl.ap(pw + ks[0]),
                                   pl.ap(pw + ks[1]))
                    for k in ks[2:-1]:
                        vec.tensor_add(pl.ap(acc), pl.ap(acc), pl.ap(pw + k))
                    vec.scalar_tensor_tensor(pl.ap(acc), pl.ap(pw + ks[-1]),
                                             float(k0), pl.ap(acc),
                                             OP.add, OP.add)
                pl.release(pw, 3)
                vec.scalar_tensor_tensor(pl.ap(y), pl.ap(y),
                                         KAPPA / (BETA * BETA), pl.ap(acc),
                                         OP.mult, OP.add)
                pl.release(acc)
                nc.sync.dma_start(out=Wm[:, ch * Tc:(ch + 1) * Tc],
                                  in_=pl.ap(y))
                pl.release(y)

            stages = [s0_load_c, s1_invar, s2_lnv, s3_dets, s4_pw_exp, s5_rc,
                      s6_ln_a, s7_h2, s8_ln_b, s9_sub, s10_xt, s11_atan,
                      s12_sin, s13_lam, s14_lnl, s15_lp, s16_exp, s17_tail]
            for stage in stages:
                for ch in range(chunks):
                    stage(ch)

            em.pin_act_order()
    nc.compile()
    return nc


def _pad_and_shard(F, T):
    """-> [NCORES, P, 9T] column-major component planes (c-major, r-minor)."""
    n = F.shape[0]
    per_core = P * T
    npad = NCORES * per_core
    flat = np.ascontiguousarray(F, dtype=np.float32).reshape(n, 9)
    if npad > n:
        pad = np.tile(np.eye(3, dtype=np.float32).reshape(1, 9), (npad - n, 1))
        flat = np.concatenate([flat, pad], axis=0)
    a = flat.reshape(NCORES, P, T, 3, 3)                 # [.., t, r, c]
    a = np.ascontiguousarray(a.transpose(0, 1, 4, 3, 2))  # [.., c, r, t]
    return a.reshape(NCORES, P, 9 * T)


def kernel(F, mu, alpha):
    F = np.asarray(F)
    n = F.shape[0]
    T = -(-n // (NCORES * P))
    T += (-T) % 4
    if T > 512:
        # keep each chunk's free dim >= 512: smaller DVE ops pay an extra
        # ~300-cycle inter-instruction bubble (measured)
        T = max(T, 1024)
    shards = _pad_and_shard(F, T)
    nc = build_nc(T, mu, alpha)
    in_maps = [{"F": shards[i]} for i in range(NCORES)]
    res = run_bass_kernel_spmd(nc, in_maps, list(range(NCORES)))
    out = np.concatenate([res.results[i]["W"].reshape(-1) for i in range(NCORES)])
    return out[:n].astype(np.float32, copy=False)


if __name__ == "__main__":
    rng = np.random.default_rng(0)
    F = np.eye(3, dtype=np.float32) + 0.1 * rng.standard_normal((4096, 3, 3)).astype(np.float32)
    mu = np.array([0.63, 0.0012, -0.01], np.float32)
    alpha = np.array([1.3, 5.0, -2.0], np.float32)
    print(kernel(F, mu, alpha)[:8])
